# revision 1
# baseline (speedup 1.0000x reference)
"""Trainium2 Bass kernel for nn_DefaultSegmentLinear.

Computes out[M, N] = (x[M, K] @ W[N, K]^T) * (s_x * s_w[chunk]) + bias[N]
with M=8192, K=4096, N=4096 (C=4 chunks of 1024 out-features).

Strategy
--------
- Host: fold the per-chunk scales into W, transpose x and W to put the
  contraction dim (K) on partitions, and cast both to bf16 (the PE
  upcasts bf16 to FP22 internally and accumulates fp32 in PSUM;
  measured rel err ~1.5e-3 vs the 2e-2 gate). bf16 halves HBM traffic
  and SBUF footprint vs fp32/fp32r at the same PE rate (the cost model
  charges 1 cycle/row for bf16 and fp32r alike; moving-row count, not
  dtype, sets the ~437us/core PE floor at 2.4 GHz).
- Sharding: M sharded 8 ways (tokens). Each core holds its x^T slice
  [K, 1024] resident in SBUF (8 MiB bf16), streams the full W^T
  [K, 4096] once (32 MiB bf16), and writes out^T [4096, 1024] fp32
  once (16 MiB): 56 MiB of HBM traffic per core, hidden under the PE
  floor (measured pure-DMA rate ~470 GB/s/core).
- Device loop: out^T is produced in 16 n-blocks of 256 columns. Each
  block accumulates 2 n-subtiles x 2 m-halves in 4 PSUM banks over all
  32 k-tiles, so consecutive blocks alternate PSUM bank sets 0-3/4-7
  and the first matmul of block nb+1 never waits on block nb's drains.
- Engine-queue roles (in-order queues; only SP/ACT/gpsimd issue DMAs):
  SP carries the W stream (2 MiB 16-k-tile groups, 4-buffer prefetch)
  plus the x/bias loads, interleaved x0-3, W(b0,g0), x4-15, W(b0,g1),
  x16-31 so a fresh pass starts its first W transfer after only 4 x
  reloads; ACT carries the PSUM drains (bias-add fused, per-partition
  scalars) and the out DMAs that depend on them. Keeping drains and W
  triggers on separate queues removes the block-boundary serialization
  that cost the fp32r baseline ~8-10%.
- Output is produced transposed ([N, M] per core); the host
  concatenates the 8 core slices and transposes back.
"""

import numpy as np
import ml_dtypes

import concourse.bacc as bacc
import concourse.mybir as mybir
import concourse.tile as tile
from concourse import bass_utils

P = 128
M, K, N = 8192, 4096, 4096
N_CORES = 8
MC = M // N_CORES           # 1024 rows of x per core
KT = K // P                 # 32 k-tiles
NB = 256                    # n-block width (2 psum banks x 2 m-halves)
NBLK = N // NB              # 16 n-blocks
NSUB = NB // P              # 2 n-subtiles per block
MHW = 512                   # moving free dim per matmul (psum bank cap)
MH = MC // MHW              # 2 m-halves
KG = 16                     # k-tiles per W-group DMA
NGRP = KT // KG             # 2 groups per n-block

F32 = mybir.dt.float32
BF16 = mybir.dt.bfloat16

_CACHE: dict = {}


def _build(iters: int = 1):
    """Build + compile the per-core Bass program.

    iters > 1 wraps the body in a hardware loop (for timing runs).
    """
    nc = bacc.Bacc("TRN2", target_bir_lowering=False, debug=False)
    xT_d = nc.dram_tensor("xT", [K, MC], BF16, kind="ExternalInput").ap()
    wT_d = nc.dram_tensor("wT", [K, N], BF16, kind="ExternalInput").ap()
    # bias pre-arranged host-side as [128, N/128]: column j holds
    # bias[j*128 : (j+1)*128] (per-partition scalars for the ACT drain).
    bias_d = nc.dram_tensor("biasc", [P, N // P], F32, kind="ExternalInput").ap()
    outT_d = nc.dram_tensor("outT", [N, MC], F32, kind="ExternalOutput").ap()

    with tile.TileContext(nc) as tc:
        with (
            tc.tile_pool(name="xres", bufs=KT) as xres_pool,
            tc.tile_pool(name="wstream", bufs=4) as w_pool,
            tc.tile_pool(name="biasp", bufs=1) as bias_pool,
            tc.tile_pool(name="ostage", bufs=8) as o_pool,
            tc.tile_pool(name="psum", bufs=8, space="PSUM") as psum_pool,
        ):
            def body(it):
                bias_sb = bias_pool.tile([P, N // P], F32, name="biassb")
                nc.sync.dma_start(bias_sb[:], bias_d[:])
                x_res = [xres_pool.tile([P, MC], BF16, tag="x",
                                        name=f"x{it}_{kt}")
                         for kt in range(KT)]

                def x_load(kts):
                    for kt in kts:
                        nc.sync.dma_start(
                            x_res[kt][:], xT_d[kt * P:(kt + 1) * P, :])

                w_tiles = {}

                def w_load(nb, g):
                    w_g = w_pool.tile([P, KG, NB], BF16, tag="w",
                                      name=f"w{it}_{nb}_{g}")
                    nc.sync.dma_start(
                        w_g[:],
                        wT_d[g * KG * P:(g + 1) * KG * P,
                             nb * NB:(nb + 1) * NB].rearrange(
                                 "(kg p) n -> p kg n", p=P))
                    w_tiles[(nb, g)] = w_g

                x_load(range(0, 4))
                w_load(0, 0)
                x_load(range(4, 16))
                w_load(0, 1)
                x_load(range(16, KT))

                for nb in range(NBLK):
                    if nb + 1 < NBLK:
                        for g in range(NGRP):
                            w_load(nb + 1, g)
                    psums = [
                        [psum_pool.tile([P, MHW], F32, tag="ps",
                                        name=f"ps{it}_{nb}_{nt}_{mh}")
                         for mh in range(MH)]
                        for nt in range(NSUB)
                    ]
                    for g in range(NGRP):
                        w_g = w_tiles.pop((nb, g))
                        for ki in range(KG):
                            kt = g * KG + ki
                            for nt in range(NSUB):
                                for mh in range(MH):
                                    nc.tensor.matmul(
                                        psums[nt][mh][:],
                                        w_g[:, ki, nt * P:(nt + 1) * P],
                                        x_res[kt][:, mh * MHW:(mh + 1) * MHW],
                                        start=(kt == 0),
                                        stop=(kt == KT - 1),
                                    )
                    for nt in range(NSUB):
                        ncol = nb * NSUB + nt
                        for mh in range(MH):
                            o_sb = o_pool.tile([P, MHW], F32, tag="o",
                                               name=f"o{it}_{nb}_{nt}_{mh}")
                            nc.scalar.activation(
                                out=o_sb[:], in_=psums[nt][mh][:],
                                func=mybir.ActivationFunctionType.Identity,
                                bias=bias_sb[:, ncol:ncol + 1],
                            )
                            nc.scalar.dma_start(
                                outT_d[ncol * P:(ncol + 1) * P,
                                       mh * MHW:(mh + 1) * MHW],
                                o_sb[:])

            if iters == 1:
                body(0)
            else:
                # For_i runs an InstAllEngineBarrier in its per-iteration
                # semaphore-reset block; unroll 4 bodies per iteration and
                # use staggered per-stage resets (one stage per body) so
                # engines never globally drain between timing iterations.
                if iters % 4 == 0:
                    with tc.For_i(0, iters // 4, 1, staggered_reset=True):
                        for u in range(4):
                            if u:
                                tc.stage_boundary()
                            body(u)
                else:
                    with tc.For_i(0, iters, 1):
                        body(0)
    nc.compile()
    return nc


def _prep_inputs(x, w_chunks, bias, input_scale, weight_scales):
    s = (np.float32(input_scale[0]) * weight_scales.astype(np.float32))
    W = w_chunks.reshape(N, K).astype(np.float32)
    W = W * np.repeat(s, N // s.shape[0]).astype(np.float32)[:, None]
    WTb = W.T.astype(ml_dtypes.bfloat16)                     # [K, N]
    xTb = x.astype(np.float32).T.astype(ml_dtypes.bfloat16)  # [K, M]
    bias_c = np.ascontiguousarray(
        bias.astype(np.float32).reshape(N // P, P).T)        # [128, N/128]
    in_maps = []
    for c in range(N_CORES):
        in_maps.append({
            "xT": np.ascontiguousarray(xTb[:, c * MC:(c + 1) * MC]),
            "wT": WTb,
            "biasc": bias_c,
        })
    return in_maps


def kernel(x, w_chunks, bias, input_scale, weight_scales):
    x = np.asarray(x)
    w_chunks = np.asarray(w_chunks)
    bias = np.asarray(bias)
    input_scale = np.asarray(input_scale)
    weight_scales = np.asarray(weight_scales)
    if "nc" not in _CACHE:
        _CACHE["nc"] = _build(iters=1)
    nc = _CACHE["nc"]
    in_maps = _prep_inputs(x, w_chunks, bias, input_scale, weight_scales)
    res = bass_utils.run_bass_kernel_spmd(
        nc, in_maps, core_ids=list(range(N_CORES)))
    outT = np.concatenate(
        [res.results[c]["outT"] for c in range(N_CORES)], axis=1)  # [N, M]
    return np.ascontiguousarray(outT.T)



# revision 8
# speedup vs baseline: 1.3304x; 1.3304x over previous
"""Trainium2 Bass kernel for nn_DefaultSegmentLinear.

Computes out[M, N] = (x[M, K] @ W[N, K]^T) * (s_x * s_w[chunk]) + bias[N]
with M=8192, K=4096, N=4096 (C=4 chunks of 1024 out-features).

Strategy
--------
- Mixed precision over the contraction dim: the first KB k-tiles run in
  bf16, the remaining KF k-tiles run in fp8 e4m3 with
  perf_mode=DoubleRow (2 MACs/cell/cycle; the PE packs two fp8 weights
  per cell, contracting 256 k per instruction). Measured rel err of the
  16/16 split is ~1.65e-2 vs the 2e-2 gate (fp8 quantization noise of
  half the k-sum; bf16 alone is 1.5e-3, pure fp8 2.4e-2).
- Weights stay UNIT scale (folding the <1 scales into fp8 weights
  would push values toward e4m3's subnormals); the per-chunk scale
  s_x*s_w[c] and the bias are applied at the PSUM drain by the ACT
  engine (out = psum*scale + bias, both per-partition operands).
- Sharding: M sharded 8 ways. Each core keeps its x^T slice resident
  in SBUF (bf16 part [KB*128, 1024] + fp8 part [KF*128, 1024] ~ 6 MiB
  at 16/16), streams W^T once (~24 MiB), writes out^T fp32 (16 MiB).
- PSUM regions are [128 n-cols, 1024 m] fp32 = 2 banks, so bf16
  matmuls use 1024-wide moving operands (the bf16/fp8 moving cap,
  halving instruction+LDWEIGHTS overhead vs 512) and fp8 DoubleRow
  matmuls share one 256-col weight load across the two 512-m halves.
  4 regions cycle through the 8 PSUM banks; consecutive n-blocks
  alternate region pairs so fresh matmuls never wait on drains.
- Output is produced transposed ([N, M] per core); the host
  concatenates the 8 core slices and transposes back.
"""

import os

import numpy as np
import ml_dtypes

import concourse.bacc as bacc
import concourse.mybir as mybir
import concourse.tile as tile
from concourse import bass_utils

P = 128
M, K, N = 8192, 4096, 4096
N_CORES = 8
MC = M // N_CORES           # 1024 rows of x per core
KT = K // P                 # 32 k-tiles
KB = int(os.environ.get("KERNEL_KB", "16"))  # bf16 k-tiles (low k)
KF = KT - KB                # fp8 k-tiles (high k)
KP = KF // 2                # fp8 DoubleRow pairs
NB = 256                    # n-block width (2 psum regions)
NBLK = N // NB              # 16 n-blocks
NSUB = NB // P              # 2 region subtiles per block

F32 = mybir.dt.float32
BF16 = mybir.dt.bfloat16
FP8 = mybir.dt.float8e4

_CACHE: dict = {}


def _build(iters: int = 1):
    """Build + compile the per-core Bass program.

    iters > 1 wraps the body in a hardware loop (for timing runs).
    """
    nc = bacc.Bacc("TRN2", target_bir_lowering=False, debug=False)
    xbT_d = nc.dram_tensor("xbT", [KB * P, MC], BF16, kind="ExternalInput").ap() \
        if KB else None
    xfT_d = nc.dram_tensor("xfT", [KF * P, MC], FP8, kind="ExternalInput").ap() \
        if KF else None
    wbT_d = nc.dram_tensor("wbT", [KB * P, N], BF16, kind="ExternalInput").ap() \
        if KB else None
    wfT_d = nc.dram_tensor("wfT", [KF * P, N], FP8, kind="ExternalInput").ap() \
        if KF else None
    # bias/scale pre-arranged host-side as [128, N/128]: column j holds
    # bias[j*128:(j+1)*128] / scale for chunk(j) (per-partition scalars
    # for the ACT drain).
    bias_d = nc.dram_tensor("biasc", [P, N // P], F32, kind="ExternalInput").ap()
    scale_d = nc.dram_tensor("scalec", [P, N // P], F32, kind="ExternalInput").ap()
    outT_d = nc.dram_tensor("outT", [N, MC], F32, kind="ExternalOutput").ap()

    with tile.TileContext(nc) as tc:
        with (
            tc.tile_pool(name="xres", bufs=max(KB, 1) + max(KP, 1)) as xres_pool,
            tc.tile_pool(name="wbstream", bufs=3) as wb_pool,
            tc.tile_pool(name="wfstream", bufs=3) as wf_pool,
            tc.tile_pool(name="biasp", bufs=2) as bias_pool,
            tc.tile_pool(name="ostage", bufs=4) as o_pool,
            tc.tile_pool(name="psum", bufs=4, space="PSUM") as psum_pool,
        ):
            def body(it):
                bias_sb = bias_pool.tile([P, N // P], F32, name=f"biassb{it}")
                nc.sync.dma_start(bias_sb[:], bias_d[:])
                scale_sb = bias_pool.tile([P, N // P], F32, name=f"scalesb{it}")
                nc.sync.dma_start(scale_sb[:], scale_d[:])
                xb_res = [xres_pool.tile([P, MC], BF16, tag="xb",
                                         name=f"xb{it}_{kt}")
                          for kt in range(KB)]
                xf_res = [xres_pool.tile([P, 2, MC], FP8, tag="xf",
                                         name=f"xf{it}_{kp}")
                          for kp in range(KP)]

                def xb_load(kts):
                    for kt in kts:
                        nc.sync.dma_start(
                            xb_res[kt][:], xbT_d[kt * P:(kt + 1) * P, :])

                def xf_load(kps):
                    for kp in kps:
                        nc.sync.dma_start(
                            xf_res[kp][:],
                            xfT_d[kp * 2 * P:(kp + 1) * 2 * P, :].rearrange(
                                "(two p) m -> p two m", p=P))

                w_tiles = {}

                def w_load(nb):
                    cols = slice(nb * NB, (nb + 1) * NB)
                    if KB:
                        w_b = wb_pool.tile([P, KB, NB], BF16, tag="wb",
                                           name=f"wb{it}_{nb}")
                        nc.sync.dma_start(
                            w_b[:],
                            wbT_d[:, cols].rearrange("(kb p) n -> p kb n", p=P))
                    else:
                        w_b = None
                    if KP:
                        w_f = wf_pool.tile([P, KP, 2, NB], FP8, tag="wf",
                                           name=f"wf{it}_{nb}")
                        nc.sync.dma_start(
                            w_f[:],
                            wfT_d[:, cols].rearrange(
                                "(kp two p) n -> p kp two n", p=P, two=2))
                    else:
                        w_f = None
                    w_tiles[nb] = (w_b, w_f)

                # Interleave the first W block into the x reloads so a
                # fresh pass starts its W stream early.
                xb_load(range(0, min(4, KB)))
                w_load(0)
                xb_load(range(4, KB))
                xf_load(range(KP))
                w_load(1)

                for nb in range(NBLK):
                    if nb + 2 < NBLK:
                        w_load(nb + 2)
                    w_b, w_f = w_tiles.pop(nb)
                    psums = [
                        psum_pool.tile([P, MC], F32, tag="ps",
                                       name=f"ps{it}_{nb}_{nt}")
                        for nt in range(NSUB)
                    ]
                    for kt in range(KB):
                        for nt in range(NSUB):
                            for mh in range(2):
                                nc.tensor.matmul(
                                    psums[nt][:, mh * 512:(mh + 1) * 512],
                                    w_b[:, kt, nt * P:(nt + 1) * P],
                                    xb_res[kt][:, mh * 512:(mh + 1) * 512],
                                    start=(kt == 0),
                                    stop=(kt == KB - 1 and KP == 0),
                                    skip_group_check=True,
                                )
                    for kp in range(KP):
                        for nt in range(NSUB):
                            for mh in range(2):
                                nc.tensor.matmul(
                                    psums[nt][:, mh * 512:(mh + 1) * 512],
                                    w_f[:, kp, :, nt * P:(nt + 1) * P],
                                    xf_res[kp][:, :, mh * 512:(mh + 1) * 512],
                                    start=(KB == 0 and kp == 0),
                                    stop=(kp == KP - 1),
                                    perf_mode=mybir.MatmulPerfMode.DoubleRow,
                                    skip_group_check=True,
                                )
                    for nt in range(NSUB):
                        ncol = nb * NSUB + nt
                        o_sb = o_pool.tile([P, MC], F32, tag="o",
                                           name=f"o{it}_{nb}_{nt}")
                        nc.scalar.activation(
                            out=o_sb[:], in_=psums[nt][:],
                            func=mybir.ActivationFunctionType.Identity,
                            bias=bias_sb[:, ncol:ncol + 1],
                            scale=scale_sb[:, ncol:ncol + 1],
                        )
                        nc.scalar.dma_start(
                            outT_d[ncol * P:(ncol + 1) * P, :], o_sb[:])

            if iters == 1:
                body(0)
            else:
                # For_i runs an InstAllEngineBarrier in its per-iteration
                # semaphore-reset block; unroll 4 bodies per iteration and
                # use staggered per-stage resets (one stage per body) so
                # engines never globally drain between timing iterations.
                if iters % 4 == 0:
                    with tc.For_i(0, iters // 4, 1, staggered_reset=True):
                        for u in range(4):
                            if u:
                                tc.stage_boundary()
                            body(u)
                else:
                    with tc.For_i(0, iters, 1):
                        body(0)
    nc.compile()
    return nc


def _prep_inputs(x, w_chunks, bias, input_scale, weight_scales):
    s = (np.float32(input_scale[0]) * weight_scales.astype(np.float32))
    scol = np.repeat(s, N // s.shape[0]).astype(np.float32)   # [N]
    W = w_chunks.reshape(N, K).astype(np.float32)             # unit scale
    WT = np.ascontiguousarray(W.T)                            # [K, N]
    xT = np.ascontiguousarray(x.astype(np.float32).T)         # [K, M]
    kb = KB * P
    wbT = WT[:kb].astype(ml_dtypes.bfloat16)
    wfT = WT[kb:].astype(ml_dtypes.float8_e4m3)
    xbT = xT[:kb].astype(ml_dtypes.bfloat16)
    xfT = xT[kb:].astype(ml_dtypes.float8_e4m3)
    bias_c = np.ascontiguousarray(
        bias.astype(np.float32).reshape(N // P, P).T)         # [128, N/128]
    scale_c = np.ascontiguousarray(
        scol.reshape(N // P, P).T)                            # [128, N/128]
    in_maps = []
    for c in range(N_CORES):
        m = {"biasc": bias_c, "scalec": scale_c}
        if KB:
            m["xbT"] = np.ascontiguousarray(xbT[:, c * MC:(c + 1) * MC])
            m["wbT"] = wbT
        if KF:
            m["xfT"] = np.ascontiguousarray(xfT[:, c * MC:(c + 1) * MC])
            m["wfT"] = wfT
        in_maps.append(m)
    return in_maps


def kernel(x, w_chunks, bias, input_scale, weight_scales):
    x = np.asarray(x)
    w_chunks = np.asarray(w_chunks)
    bias = np.asarray(bias)
    input_scale = np.asarray(input_scale)
    weight_scales = np.asarray(weight_scales)
    if "nc" not in _CACHE:
        _CACHE["nc"] = _build(iters=1)
    nc = _CACHE["nc"]
    in_maps = _prep_inputs(x, w_chunks, bias, input_scale, weight_scales)
    res = bass_utils.run_bass_kernel_spmd(
        nc, in_maps, core_ids=list(range(N_CORES)))
    outT = np.concatenate(
        [res.results[c]["outT"] for c in range(N_CORES)], axis=1)  # [N, M]
    return np.ascontiguousarray(outT.T)
